# revision 14
# baseline (speedup 1.0000x reference)
"""LoRA Linear (y = x @ W^T + bias + x @ (B@A)^T) on 8 Trainium2 NeuronCores.

Strategy (hybrid shard: tokens 4-way x out_features 2-way, all-bf16 GEMM):
  - Core c owns tokens [t_grp*2048, +2048) and outputs [o_grp*2048, +2048)
    with t_grp = c // 2, o_grp = c % 2. No collectives; host assembles the
    4x2 output grid.
  - bf16 on the PE: matmul streams at 1 col/cycle (213ns per 128x128x512),
    LDWEIGHTS gets FWL, and a post-schedule pass drops the redundant
    weight reloads the legalizer emits, so each stationary x-tile is
    loaded once for its 4 out-chunk matmuls (measured 216ns/MM sustained).
  - LoRA is applied as a rank-16 closing matmul into the same PSUM
    accumulation: psum[t] = sum_k x_k^T @ W_k + u_t^T @ B^T, with
    u^T = A @ x^T ([16 x tokens], ~0.5MB) precomputed on the host during
    input packing. This avoids materializing delta_W = B@A (8.4M elems
    per core whose PSUM eviction cost ~75us and kept the tensor engine
    throttled through the first ~95us in the fold-on-device variant).
  - Warmup: ~16 full-width matmuls on the first x tile ramp the HAM
    p-state during the W-DMA window, so the main GEMM runs at the warm
    clock from the start.
  - PSUM: 4 chunk tags x 2 bufs = all 8 banks (4 accumulate, 4 drain).

Host-side work: pack x as [p, T, a, t] bf16, pre-transpose W/B to bf16,
u = x @ A^T (f32 GEMM, then bf16), broadcast bias; assemble the output.
"""

import numpy as np

B_DIM, S_DIM = 4, 2048
IN_F = 4096
OUT_F = 4096
RANK = 16
N_CORES = 8
T_GRPS = 4                          # token groups
O_GRPS = 2                          # out_features groups
TOK = B_DIM * S_DIM                 # 8192
TOK_SHARD = TOK // T_GRPS           # 2048 tokens per core
O_SHARD = OUT_F // O_GRPS           # 2048 outs per core
T_TILES = TOK_SHARD // 128          # 16
K_TILES = IN_F // 128               # 32
OC = O_SHARD // 512                 # 4 out chunks of 512 (one PSUM bank)
N_XPRE = 3                          # x tiles DMA'd before the W stream
N_WARMUP = 40                       # HAM ramp matmuls bridging to x0+w0

_CACHE = {}
LAST_RESULTS = None  # test harness introspection


def _dedup_ldweights(nc, mybir):
    """Drop InstLdweights whose weights AP matches the immediately
    preceding weight load (the legalizer emits one per matmul even when
    consecutive matmuls share the stationary operand). Any sync carried
    by a dropped load is pushed onto the next matmul."""
    removed = 0
    for blk in nc.main_func.blocks:
        insts = blk.instructions
        out = []
        last_sig = None
        pending = []
        for inst in insts:
            if isinstance(inst, mybir.InstLdweights):
                sig = (str(inst.ins[0]),
                       str(getattr(inst, "perf_mode", None)),
                       str(getattr(inst, "is_transpose", None)),
                       str(getattr(inst, "tile_position", None)),
                       str(getattr(inst, "tile_size", None)))
                if sig == last_sig:
                    si = inst.sync_info
                    if si is not None and (len(si.on_wait) or len(si.on_update)):
                        pending.append(si)
                    removed += 1
                    continue
                last_sig = sig
                out.append(inst)
            elif isinstance(inst, mybir.InstMatmult) and pending:
                si = inst.sync_info
                waits = [w for p in pending for w in p.on_wait]
                ups = [u for p in pending for u in p.on_update]
                if si is None:
                    inst.sync_info = mybir.SyncInfo(on_wait=waits,
                                                    on_update=ups)
                else:
                    si.on_wait = list(si.on_wait) + waits
                    si.on_update = list(si.on_update) + ups
                pending = []
                out.append(inst)
            else:
                out.append(inst)
        assert not pending
        insts[:] = out
    return removed


def _build_nc():
    import concourse.mybir as mybir
    import concourse.tile as tile
    from concourse import bacc

    nc = bacc.Bacc("TRN2", target_bir_lowering=False)
    f32 = mybir.dt.float32
    bf16 = mybir.dt.bfloat16

    x_d = nc.dram_tensor("x_re", (128, T_TILES, K_TILES, 128), bf16,
                         kind="ExternalInput")
    w_d = nc.dram_tensor("w_re", (128, K_TILES, O_SHARD), bf16,
                         kind="ExternalInput")
    u_d = nc.dram_tensor("u_t", (RANK, TOK_SHARD), bf16,
                         kind="ExternalInput")
    bt_d = nc.dram_tensor("b_t", (RANK, O_SHARD), bf16, kind="ExternalInput")
    bias_d = nc.dram_tensor("bias_b", (128, O_SHARD), f32,
                            kind="ExternalInput")
    y_d = nc.dram_tensor("y", (TOK_SHARD, O_SHARD), f32, kind="ExternalOutput")

    with tile.TileContext(nc) as tc:
        with (
            tc.tile_pool(name="wpool", bufs=1) as wpool,
            tc.tile_pool(name="const", bufs=1) as const,
            tc.tile_pool(name="xpool", bufs=3) as xpool,
            tc.tile_pool(name="opool", bufs=2) as opool,
            tc.tile_pool(name="psum", bufs=2, space="PSUM") as psum_pool,
        ):
            # Tiny dedicated warmup tile first in the DMA queue (~160KB,
            # lands in ~1.5us) so the PE can start ramping immediately.
            xw = const.tile([128, 5, 128], bf16)
            nc.sync.dma_start(xw[:], x_d[:, 0, 0:5, :])

            x_tiles = {}
            for t in range(N_XPRE):
                x_sb = xpool.tile([128, K_TILES, 128], bf16)
                nc.sync.dma_start(x_sb[:], x_d[:, t, :, :])
                x_tiles[t] = x_sb

            u_sb = const.tile([RANK, TOK_SHARD], bf16)
            nc.sync.dma_start(u_sb[:], u_d[:])
            b_sb = const.tile([RANK, O_SHARD], bf16)
            nc.sync.dma_start(b_sb[:], bt_d[:])
            bias_sb = const.tile([128, O_SHARD], f32)
            nc.sync.dma_start(bias_sb[:], bias_d[:])

            # HAM warmup: full-width matmuls into a discarded PSUM tile.
            # The p-state ladder reaches the 2.4GHz warm clock only after
            # ~3.4us of continuous full-width streaming, and any PE idle
            # gap re-throttles it; these bridge the window until x0 and
            # the first W tiles have landed, so the main GEMM starts warm.
            wu = psum_pool.tile([128, 512], f32, tag="pt0")
            for _ in range(N_WARMUP):
                nc.tensor.matmul(wu[:], xw[:, 0, :], xw[:, 1:5, :],
                                 start=True, stop=True)

            # W stream: raw weights, no on-device fold.
            w_sb = []
            for k in range(K_TILES):
                w_t = wpool.tile([128, O_SHARD], bf16, tag=f"w{k}")
                nc.sync.dma_start(w_t[:], w_d[:, k, :])
                w_sb.append(w_t)

            # Main GEMM: psum[c][128t, 512o] = sum_k x_tile_k^T @ W_k
            #                                  + u_t^T @ B^T   (rank-16)
            # The stationary x-tile serves the 4 out-chunk matmuls off one
            # weight load; the LoRA close rides the same accumulation.
            # The first two token tiles run with their k-loops interleaved
            # (both psum ring slots live) so the W-stream window is filled
            # with matmul work instead of leaving the PE ~40% idle (and
            # re-throttling the clock) while t=0 alone paces with the DMA.
            def x_tile_for(t):
                if t in x_tiles:
                    return x_tiles.pop(t)
                x_sb = xpool.tile([128, K_TILES, 128], bf16, name="x_sb")
                nc.sync.dma_start(x_sb[:], x_d[:, t, :, :])
                return x_sb

            def close_and_evict(t, pts):
                for c in range(OC):
                    nc.tensor.matmul(
                        pts[c][:],
                        u_sb[:, t * 128:(t + 1) * 128],
                        b_sb[:, c * 512:(c + 1) * 512],
                        start=False, stop=True,
                    )
                o_sb = opool.tile([128, O_SHARD], f32, name="o_sb")
                for c in range(OC):
                    nc.vector.tensor_add(
                        o_sb[:, c * 512:(c + 1) * 512],
                        pts[c][:],
                        bias_sb[:, c * 512:(c + 1) * 512],
                    )
                nc.sync.dma_start(y_d[t * 128:(t + 1) * 128, :], o_sb[:])

            def new_pts(t):
                return [psum_pool.tile([128, 512], f32, name="pt",
                                       tag=f"pt{c}")
                        for c in range(OC)]

            x0_sb, x1_sb = x_tile_for(0), x_tile_for(1)
            pts0, pts1 = new_pts(0), new_pts(1)
            for k in range(K_TILES):
                for x_sb, pts in ((x0_sb, pts0), (x1_sb, pts1)):
                    for c in range(OC):
                        nc.tensor.matmul(
                            pts[c][:],
                            x_sb[:, k, :],
                            w_sb[k][:, c * 512:(c + 1) * 512],
                            start=(k == 0), stop=False,
                        )
            close_and_evict(0, pts0)
            close_and_evict(1, pts1)

            for t in range(2, T_TILES):
                x_sb = x_tile_for(t)
                pts = new_pts(t)
                for k in range(K_TILES):
                    for c in range(OC):
                        nc.tensor.matmul(
                            pts[c][:],
                            x_sb[:, k, :],
                            w_sb[k][:, c * 512:(c + 1) * 512],
                            start=(k == 0), stop=False,
                        )
                close_and_evict(t, pts)

    removed = _dedup_ldweights(nc, mybir)
    assert removed > 1500, f"ldweights dedup removed only {removed}"
    nc.compile()
    return nc


def _pack_x(x_f32):
    import ml_dtypes
    # x_re[p, T, a, t] = x2[T*128 + t, a*128 + p], per token group
    out = []
    for g in range(T_GRPS):
        x2 = x_f32[g * TOK_SHARD:(g + 1) * TOK_SHARD]
        xr = x2.reshape(T_TILES, 128, K_TILES, 128)      # (T, t, a, p)
        out.append(np.ascontiguousarray(
            xr.transpose(3, 0, 2, 1).astype(ml_dtypes.bfloat16)))
    return out


def kernel(x, weight, A, B, bias):
    global LAST_RESULTS
    import ml_dtypes
    from concourse.bass_utils import run_bass_kernel_spmd

    if "nc" not in _CACHE:
        _CACHE["nc"] = _build_nc()
    nc = _CACHE["nc"]

    weight = np.asarray(weight, dtype=np.float32)
    A = np.asarray(A, dtype=np.float32)
    B = np.asarray(B, dtype=np.float32)
    bias = np.asarray(bias, dtype=np.float32)
    x2 = np.asarray(x, dtype=np.float32).reshape(TOK, IN_F)

    x_parts = _pack_x(x2)
    # u^T = A @ x^T, [16 x TOK] in f32, sliced per token group in bf16
    u_t = A @ x2.T
    u_parts = [np.ascontiguousarray(
        u_t[:, g * TOK_SHARD:(g + 1) * TOK_SHARD].astype(ml_dtypes.bfloat16))
        for g in range(T_GRPS)]

    w_parts, b_parts, bias_parts = [], [], []
    for g in range(O_GRPS):
        sl = slice(g * O_SHARD, (g + 1) * O_SHARD)
        w_s = weight[sl]                                  # (2048, 4096)
        # w_re[p, a, o] = w_s[o, a*128 + p]
        w_parts.append(np.ascontiguousarray(
            w_s.T.reshape(K_TILES, 128, O_SHARD).transpose(1, 0, 2)
            .astype(ml_dtypes.bfloat16)))
        b_parts.append(np.ascontiguousarray(
            B[sl].T.astype(ml_dtypes.bfloat16)))          # (16, 2048)
        bias_parts.append(np.ascontiguousarray(
            np.broadcast_to(bias[sl], (128, O_SHARD))))

    in_maps = []
    for core in range(N_CORES):
        t_grp, o_grp = core // O_GRPS, core % O_GRPS
        in_maps.append({
            "x_re": x_parts[t_grp],
            "w_re": w_parts[o_grp],
            "u_t": u_parts[t_grp],
            "b_t": b_parts[o_grp],
            "bias_b": bias_parts[o_grp],
        })

    res = run_bass_kernel_spmd(nc, in_maps, core_ids=list(range(N_CORES)))
    LAST_RESULTS = res

    y = np.empty((TOK, OUT_F), dtype=np.float32)
    for core in range(N_CORES):
        t_grp, o_grp = core // O_GRPS, core % O_GRPS
        y[t_grp * TOK_SHARD:(t_grp + 1) * TOK_SHARD,
          o_grp * O_SHARD:(o_grp + 1) * O_SHARD] = res.results[core]["y"]
    return y.reshape(B_DIM, S_DIM, OUT_F)


# revision 17
# speedup vs baseline: 1.0060x; 1.0060x over previous
"""LoRA Linear (y = x @ W^T + bias + x @ (B@A)^T) on 8 Trainium2 NeuronCores.

Strategy (hybrid shard: tokens 4-way x out_features 2-way, all-bf16 GEMM):
  - Core c owns tokens [t_grp*2048, +2048) and outputs [o_grp*2048, +2048)
    with t_grp = c // 2, o_grp = c % 2. No collectives; host assembles the
    4x2 output grid.
  - bf16 on the PE: matmul streams at 1 col/cycle (213ns per 128x128x512),
    LDWEIGHTS gets FWL, and a post-schedule pass drops the redundant
    weight reloads the legalizer emits, so each stationary x-tile is
    loaded once for its 4 out-chunk matmuls (measured 216ns/MM sustained).
  - LoRA is applied as a rank-16 closing matmul into the same PSUM
    accumulation: psum[t] = sum_k x_k^T @ W_k + u_t^T @ B^T, with
    u^T = A @ x^T ([16 x tokens], ~0.5MB) precomputed on the host during
    input packing. This avoids materializing delta_W = B@A (8.4M elems
    per core whose PSUM eviction cost ~75us and kept the tensor engine
    throttled through the first ~95us in the fold-on-device variant).
  - Warmup: ~16 full-width matmuls on the first x tile ramp the HAM
    p-state during the W-DMA window, so the main GEMM runs at the warm
    clock from the start.
  - PSUM: 4 chunk tags x 2 bufs = all 8 banks (4 accumulate, 4 drain).

Host-side work: pack x as [p, T, a, t] bf16, pre-transpose W/B to bf16,
u = x @ A^T (f32 GEMM, then bf16), broadcast bias; assemble the output.
"""

import numpy as np

B_DIM, S_DIM = 4, 2048
IN_F = 4096
OUT_F = 4096
RANK = 16
N_CORES = 8
T_GRPS = 4                          # token groups
O_GRPS = 2                          # out_features groups
TOK = B_DIM * S_DIM                 # 8192
TOK_SHARD = TOK // T_GRPS           # 2048 tokens per core
O_SHARD = OUT_F // O_GRPS           # 2048 outs per core
T_TILES = TOK_SHARD // 128          # 16
K_TILES = IN_F // 128               # 32
OC = O_SHARD // 512                 # 4 out chunks of 512 (one PSUM bank)
N_XPRE = 2                          # x tiles DMA'd before the W stream
N_WARMUP = 72                       # HAM ramp matmuls bridging to x0+w0

_CACHE = {}
LAST_RESULTS = None  # test harness introspection


def _dedup_ldweights(nc, mybir):
    """Drop InstLdweights whose weights AP matches the immediately
    preceding weight load (the legalizer emits one per matmul even when
    consecutive matmuls share the stationary operand). Any sync carried
    by a dropped load is pushed onto the next matmul."""
    removed = 0
    for blk in nc.main_func.blocks:
        insts = blk.instructions
        out = []
        last_sig = None
        pending = []
        for inst in insts:
            if isinstance(inst, mybir.InstLdweights):
                sig = (str(inst.ins[0]),
                       str(getattr(inst, "perf_mode", None)),
                       str(getattr(inst, "is_transpose", None)),
                       str(getattr(inst, "tile_position", None)),
                       str(getattr(inst, "tile_size", None)))
                if sig == last_sig:
                    si = inst.sync_info
                    if si is not None and (len(si.on_wait) or len(si.on_update)):
                        pending.append(si)
                    removed += 1
                    continue
                last_sig = sig
                out.append(inst)
            elif isinstance(inst, mybir.InstMatmult) and pending:
                si = inst.sync_info
                waits = [w for p in pending for w in p.on_wait]
                ups = [u for p in pending for u in p.on_update]
                if si is None:
                    inst.sync_info = mybir.SyncInfo(on_wait=waits,
                                                    on_update=ups)
                else:
                    si.on_wait = list(si.on_wait) + waits
                    si.on_update = list(si.on_update) + ups
                pending = []
                out.append(inst)
            else:
                out.append(inst)
        assert not pending
        insts[:] = out
    return removed


def _build_nc():
    import concourse.mybir as mybir
    import concourse.tile as tile
    from concourse import bacc

    nc = bacc.Bacc("TRN2", target_bir_lowering=False)
    f32 = mybir.dt.float32
    bf16 = mybir.dt.bfloat16

    x_d = nc.dram_tensor("x_re", (128, T_TILES, K_TILES, 128), bf16,
                         kind="ExternalInput")
    w_d = nc.dram_tensor("w_re", (128, K_TILES, O_SHARD), bf16,
                         kind="ExternalInput")
    u_d = nc.dram_tensor("u_t", (RANK, TOK_SHARD), bf16,
                         kind="ExternalInput")
    bt_d = nc.dram_tensor("b_t", (RANK, O_SHARD), bf16, kind="ExternalInput")
    bias_d = nc.dram_tensor("bias_b", (128, O_SHARD), f32,
                            kind="ExternalInput")
    y_d = nc.dram_tensor("y", (TOK_SHARD, O_SHARD), f32, kind="ExternalOutput")

    with tile.TileContext(nc) as tc:
        with (
            tc.tile_pool(name="wpool", bufs=1) as wpool,
            tc.tile_pool(name="const", bufs=1) as const,
            tc.tile_pool(name="xpool", bufs=3) as xpool,
            tc.tile_pool(name="opool", bufs=2) as opool,
            tc.tile_pool(name="psum", bufs=2, space="PSUM") as psum_pool,
        ):
            # Warmup scratch: zeroed on-chip, no DMA dependency, so the
            # PE can start ramping ~1us in (the first DMA takes ~10us to
            # land regardless of size).
            xw = const.tile([128, 5, 128], bf16)
            nc.scalar.memzero(xw[:])

            x_tiles = {}
            for t in range(N_XPRE):
                x_sb = xpool.tile([128, K_TILES, 128], bf16)
                nc.sync.dma_start(x_sb[:], x_d[:, t, :, :])
                x_tiles[t] = x_sb

            u_sb = const.tile([RANK, TOK_SHARD], bf16)
            nc.sync.dma_start(u_sb[:], u_d[:])
            b_sb = const.tile([RANK, O_SHARD], bf16)
            nc.sync.dma_start(b_sb[:], bt_d[:])
            bias_sb = const.tile([128, O_SHARD], f32)
            nc.sync.dma_start(bias_sb[:], bias_d[:])

            # HAM warmup: full-width matmuls into a discarded PSUM tile.
            # The p-state ladder reaches the 2.4GHz warm clock only after
            # ~3.4us of continuous full-width streaming, and any PE idle
            # gap re-throttles it; these bridge the window until x0 and
            # the first W tiles have landed, so the main GEMM starts warm.
            wu = psum_pool.tile([128, 512], f32, tag="pt0")
            for _ in range(N_WARMUP):
                nc.tensor.matmul(wu[:], xw[:, 0, :], xw[:, 1:5, :],
                                 start=True, stop=True)

            # W stream: raw weights, no on-device fold.
            w_sb = []
            for k in range(K_TILES):
                w_t = wpool.tile([128, O_SHARD], bf16, tag=f"w{k}")
                nc.sync.dma_start(w_t[:], w_d[:, k, :])
                w_sb.append(w_t)

            # Main GEMM: psum[c][128t, 512o] = sum_k x_tile_k^T @ W_k
            #                                  + u_t^T @ B^T   (rank-16)
            # The stationary x-tile serves the 4 out-chunk matmuls off one
            # weight load; the LoRA close rides the same accumulation.
            # The first two token tiles run with their k-loops interleaved
            # (both psum ring slots live) so the W-stream window is filled
            # with matmul work instead of leaving the PE ~40% idle (and
            # re-throttling the clock) while t=0 alone paces with the DMA.
            def x_tile_for(t):
                if t in x_tiles:
                    return x_tiles.pop(t)
                x_sb = xpool.tile([128, K_TILES, 128], bf16, name="x_sb")
                nc.sync.dma_start(x_sb[:], x_d[:, t, :, :])
                return x_sb

            def close_and_evict(t, pts):
                # Per-chunk close -> bias-add -> DMA so the eviction of
                # chunk c overlaps the closes of c+1.. (trims the exposed
                # tail after the last matmul).
                o_sb = opool.tile([128, O_SHARD], f32, name="o_sb")
                for c in range(OC):
                    sl = slice(c * 512, (c + 1) * 512)
                    nc.tensor.matmul(
                        pts[c][:],
                        u_sb[:, t * 128:(t + 1) * 128],
                        b_sb[:, sl],
                        start=False, stop=True,
                    )
                    nc.vector.tensor_add(o_sb[:, sl], pts[c][:],
                                         bias_sb[:, sl])
                    nc.sync.dma_start(y_d[t * 128:(t + 1) * 128, sl],
                                      o_sb[:, sl])

            def new_pts(t):
                return [psum_pool.tile([128, 512], f32, name="pt",
                                       tag=f"pt{c}")
                        for c in range(OC)]

            x0_sb, x1_sb = x_tile_for(0), x_tile_for(1)
            pts0, pts1 = new_pts(0), new_pts(1)
            for k in range(K_TILES):
                for x_sb, pts in ((x0_sb, pts0), (x1_sb, pts1)):
                    for c in range(OC):
                        nc.tensor.matmul(
                            pts[c][:],
                            x_sb[:, k, :],
                            w_sb[k][:, c * 512:(c + 1) * 512],
                            start=(k == 0), stop=False,
                        )
            close_and_evict(0, pts0)
            close_and_evict(1, pts1)

            for t in range(2, T_TILES):
                x_sb = x_tile_for(t)
                pts = new_pts(t)
                for k in range(K_TILES):
                    for c in range(OC):
                        nc.tensor.matmul(
                            pts[c][:],
                            x_sb[:, k, :],
                            w_sb[k][:, c * 512:(c + 1) * 512],
                            start=(k == 0), stop=False,
                        )
                close_and_evict(t, pts)

    removed = _dedup_ldweights(nc, mybir)
    assert removed > 1500, f"ldweights dedup removed only {removed}"
    nc.compile()
    return nc


def _pack_x(x_f32):
    import ml_dtypes
    # x_re[p, T, a, t] = x2[T*128 + t, a*128 + p], per token group
    out = []
    for g in range(T_GRPS):
        x2 = x_f32[g * TOK_SHARD:(g + 1) * TOK_SHARD]
        xr = x2.reshape(T_TILES, 128, K_TILES, 128)      # (T, t, a, p)
        out.append(np.ascontiguousarray(
            xr.transpose(3, 0, 2, 1).astype(ml_dtypes.bfloat16)))
    return out


def kernel(x, weight, A, B, bias):
    global LAST_RESULTS
    import ml_dtypes
    from concourse.bass_utils import run_bass_kernel_spmd

    if "nc" not in _CACHE:
        _CACHE["nc"] = _build_nc()
    nc = _CACHE["nc"]

    weight = np.asarray(weight, dtype=np.float32)
    A = np.asarray(A, dtype=np.float32)
    B = np.asarray(B, dtype=np.float32)
    bias = np.asarray(bias, dtype=np.float32)
    x2 = np.asarray(x, dtype=np.float32).reshape(TOK, IN_F)

    x_parts = _pack_x(x2)
    # u^T = A @ x^T, [16 x TOK] in f32, sliced per token group in bf16
    u_t = A @ x2.T
    u_parts = [np.ascontiguousarray(
        u_t[:, g * TOK_SHARD:(g + 1) * TOK_SHARD].astype(ml_dtypes.bfloat16))
        for g in range(T_GRPS)]

    w_parts, b_parts, bias_parts = [], [], []
    for g in range(O_GRPS):
        sl = slice(g * O_SHARD, (g + 1) * O_SHARD)
        w_s = weight[sl]                                  # (2048, 4096)
        # w_re[p, a, o] = w_s[o, a*128 + p]
        w_parts.append(np.ascontiguousarray(
            w_s.T.reshape(K_TILES, 128, O_SHARD).transpose(1, 0, 2)
            .astype(ml_dtypes.bfloat16)))
        b_parts.append(np.ascontiguousarray(
            B[sl].T.astype(ml_dtypes.bfloat16)))          # (16, 2048)
        bias_parts.append(np.ascontiguousarray(
            np.broadcast_to(bias[sl], (128, O_SHARD))))

    in_maps = []
    for core in range(N_CORES):
        t_grp, o_grp = core // O_GRPS, core % O_GRPS
        in_maps.append({
            "x_re": x_parts[t_grp],
            "w_re": w_parts[o_grp],
            "u_t": u_parts[t_grp],
            "b_t": b_parts[o_grp],
            "bias_b": bias_parts[o_grp],
        })

    res = run_bass_kernel_spmd(nc, in_maps, core_ids=list(range(N_CORES)))
    LAST_RESULTS = res

    y = np.empty((TOK, OUT_F), dtype=np.float32)
    for core in range(N_CORES):
        t_grp, o_grp = core // O_GRPS, core % O_GRPS
        y[t_grp * TOK_SHARD:(t_grp + 1) * TOK_SHARD,
          o_grp * O_SHARD:(o_grp + 1) * O_SHARD] = res.results[core]["y"]
    return y.reshape(B_DIM, S_DIM, OUT_F)


# revision 19
# speedup vs baseline: 1.0259x; 1.0198x over previous
"""LoRA Linear (y = x @ W^T + bias + x @ (B@A)^T) on 8 Trainium2 NeuronCores.

Strategy (hybrid shard: tokens 4-way x out_features 2-way, all-bf16 GEMM):
  - Core c owns tokens [t_grp*2048, +2048) and outputs [o_grp*2048, +2048)
    with t_grp = c // 2, o_grp = c % 2. No collectives; host assembles the
    4x2 output grid.
  - bf16 on the PE: matmul streams at 1 col/cycle (213ns per 128x128x512),
    LDWEIGHTS gets FWL, and a post-schedule pass drops the redundant
    weight reloads the legalizer emits, so each stationary x-tile is
    loaded once for its 4 out-chunk matmuls (measured 216ns/MM sustained).
  - LoRA is applied as a rank-16 closing matmul into the same PSUM
    accumulation: psum[t] = sum_k x_k^T @ W_k + u_t^T @ B^T, with
    u^T = A @ x^T ([16 x tokens], ~0.5MB) precomputed on the host during
    input packing. This avoids materializing delta_W = B@A (8.4M elems
    per core whose PSUM eviction cost ~75us and kept the tensor engine
    throttled through the first ~95us in the fold-on-device variant).
  - Warmup: ~16 full-width matmuls on the first x tile ramp the HAM
    p-state during the W-DMA window, so the main GEMM runs at the warm
    clock from the start.
  - PSUM: 4 chunk tags x 2 bufs = all 8 banks (4 accumulate, 4 drain).

Host-side work: pack x as [p, T, a, t] bf16, pre-transpose W/B to bf16,
u = x @ A^T (f32 GEMM, then bf16), broadcast bias; assemble the output.
"""

import numpy as np

B_DIM, S_DIM = 4, 2048
IN_F = 4096
OUT_F = 4096
RANK = 16
N_CORES = 8
T_GRPS = 4                          # token groups
O_GRPS = 2                          # out_features groups
TOK = B_DIM * S_DIM                 # 8192
TOK_SHARD = TOK // T_GRPS           # 2048 tokens per core
O_SHARD = OUT_F // O_GRPS           # 2048 outs per core
T_TILES = TOK_SHARD // 128          # 16
K_TILES = IN_F // 128               # 32
OC = O_SHARD // 512                 # 4 out chunks of 512 (one PSUM bank)
N_XPRE = 2                          # x tiles DMA'd before the W stream
N_WARMUP = 28                       # HAM ramp matmuls bridging to x0+w0

_CACHE = {}
LAST_RESULTS = None  # test harness introspection


def _dedup_ldweights(nc, mybir):
    """Drop InstLdweights whose weights AP matches the immediately
    preceding weight load (the legalizer emits one per matmul even when
    consecutive matmuls share the stationary operand). Any sync carried
    by a dropped load is pushed onto the next matmul."""
    removed = 0
    for blk in nc.main_func.blocks:
        insts = blk.instructions
        out = []
        last_sig = None
        pending = []
        for inst in insts:
            if isinstance(inst, mybir.InstLdweights):
                sig = (str(inst.ins[0]),
                       str(getattr(inst, "perf_mode", None)),
                       str(getattr(inst, "is_transpose", None)),
                       str(getattr(inst, "tile_position", None)),
                       str(getattr(inst, "tile_size", None)))
                if sig == last_sig:
                    si = inst.sync_info
                    if si is not None and (len(si.on_wait) or len(si.on_update)):
                        pending.append(si)
                    removed += 1
                    continue
                last_sig = sig
                out.append(inst)
            elif isinstance(inst, mybir.InstMatmult) and pending:
                si = inst.sync_info
                waits = [w for p in pending for w in p.on_wait]
                ups = [u for p in pending for u in p.on_update]
                if si is None:
                    inst.sync_info = mybir.SyncInfo(on_wait=waits,
                                                    on_update=ups)
                else:
                    si.on_wait = list(si.on_wait) + waits
                    si.on_update = list(si.on_update) + ups
                pending = []
                out.append(inst)
            else:
                out.append(inst)
        assert not pending
        insts[:] = out
    return removed


def _build_nc():
    import concourse.mybir as mybir
    import concourse.tile as tile
    from concourse import bacc

    nc = bacc.Bacc("TRN2", target_bir_lowering=False)
    f32 = mybir.dt.float32
    bf16 = mybir.dt.bfloat16

    x_d = nc.dram_tensor("x_re", (128, T_TILES, K_TILES, 128), bf16,
                         kind="ExternalInput")
    w_d = nc.dram_tensor("w_re", (128, K_TILES, O_SHARD), bf16,
                         kind="ExternalInput")
    u_d = nc.dram_tensor("u_t", (RANK, TOK_SHARD), bf16,
                         kind="ExternalInput")
    bt_d = nc.dram_tensor("b_t", (RANK, O_SHARD), bf16, kind="ExternalInput")
    bias_d = nc.dram_tensor("bias_b", (128, O_SHARD), f32,
                            kind="ExternalInput")
    y_d = nc.dram_tensor("y", (TOK_SHARD, O_SHARD), f32, kind="ExternalOutput")

    with tile.TileContext(nc) as tc:
        with (
            tc.tile_pool(name="wpool", bufs=1) as wpool,
            tc.tile_pool(name="const", bufs=1) as const,
            tc.tile_pool(name="xpool", bufs=3) as xpool,
            tc.tile_pool(name="opool", bufs=2) as opool,
            tc.tile_pool(name="psum", bufs=2, space="PSUM") as psum_pool,
        ):
            # Warmup scratch: zeroed on-chip, no DMA dependency, so the
            # PE can start ramping ~1us in (the first DMA takes ~10us to
            # land regardless of size).
            xw = const.tile([128, 5, 128], bf16)
            nc.scalar.memzero(xw[:])

            # DMA order: x0, x1, then the W stream; the tiny consts
            # (u/B/bias, ~0.25MB) ride after W — they are first needed at
            # t=0's close, ~10us after the W stream finishes.
            x_tiles = {}
            for t in range(N_XPRE):
                x_sb = xpool.tile([128, K_TILES, 128], bf16)
                nc.sync.dma_start(x_sb[:], x_d[:, t, :, :])
                x_tiles[t] = x_sb

            # W stream: raw weights, no on-device fold.
            w_sb = []
            for k in range(K_TILES):
                w_t = wpool.tile([128, O_SHARD], bf16, tag=f"w{k}")
                nc.sync.dma_start(w_t[:], w_d[:, k, :])
                w_sb.append(w_t)

            u_sb = const.tile([RANK, TOK_SHARD], bf16)
            nc.sync.dma_start(u_sb[:], u_d[:])
            b_sb = const.tile([RANK, O_SHARD], bf16)
            nc.sync.dma_start(b_sb[:], bt_d[:])
            bias_sb = const.tile([128, O_SHARD], f32)
            nc.sync.dma_start(bias_sb[:], bias_d[:])

            # HAM warmup: full-width matmuls into a discarded PSUM tile.
            # The p-state ladder reaches the 2.4GHz warm clock only after
            # ~3.4us of continuous full-width streaming, and any PE idle
            # gap >~1.7us re-throttles it; these bridge the window until
            # x0 and the first W tiles land, so the main GEMM starts warm.
            wu = psum_pool.tile([128, 512], f32, tag="pt0")
            for _ in range(N_WARMUP):
                nc.tensor.matmul(wu[:], xw[:, 0, :], xw[:, 1:5, :],
                                 start=True, stop=True)

            # Main GEMM: psum[c][128t, 512o] = sum_k x_tile_k^T @ W_k
            #                                  + u_t^T @ B^T   (rank-16)
            # The stationary x-tile serves the 4 out-chunk matmuls off one
            # weight load; the LoRA close rides the same accumulation.
            # The first two token tiles run with their k-loops interleaved
            # (both psum ring slots live) so the W-stream window is filled
            # with matmul work instead of leaving the PE ~40% idle (and
            # re-throttling the clock) while t=0 alone paces with the DMA.
            def x_tile_for(t):
                if t in x_tiles:
                    return x_tiles.pop(t)
                x_sb = xpool.tile([128, K_TILES, 128], bf16, name="x_sb")
                nc.sync.dma_start(x_sb[:], x_d[:, t, :, :])
                return x_sb

            def close_and_evict(t, pts):
                # Per-chunk close -> bias-add -> DMA so the eviction of
                # chunk c overlaps the closes of c+1.. (trims the exposed
                # tail after the last matmul).
                o_sb = opool.tile([128, O_SHARD], f32, name="o_sb")
                for c in range(OC):
                    sl = slice(c * 512, (c + 1) * 512)
                    nc.tensor.matmul(
                        pts[c][:],
                        u_sb[:, t * 128:(t + 1) * 128],
                        b_sb[:, sl],
                        start=False, stop=True,
                    )
                    nc.vector.tensor_add(o_sb[:, sl], pts[c][:],
                                         bias_sb[:, sl])
                    nc.sync.dma_start(y_d[t * 128:(t + 1) * 128, sl],
                                      o_sb[:, sl])

            def new_pts(t):
                return [psum_pool.tile([128, 512], f32, name="pt",
                                       tag=f"pt{c}")
                        for c in range(OC)]

            x0_sb, x1_sb = x_tile_for(0), x_tile_for(1)
            pts0, pts1 = new_pts(0), new_pts(1)
            for k in range(K_TILES):
                for x_sb, pts in ((x0_sb, pts0), (x1_sb, pts1)):
                    for c in range(OC):
                        nc.tensor.matmul(
                            pts[c][:],
                            x_sb[:, k, :],
                            w_sb[k][:, c * 512:(c + 1) * 512],
                            start=(k == 0), stop=False,
                        )
            close_and_evict(0, pts0)
            close_and_evict(1, pts1)

            for t in range(2, T_TILES):
                x_sb = x_tile_for(t)
                pts = new_pts(t)
                for k in range(K_TILES):
                    for c in range(OC):
                        nc.tensor.matmul(
                            pts[c][:],
                            x_sb[:, k, :],
                            w_sb[k][:, c * 512:(c + 1) * 512],
                            start=(k == 0), stop=False,
                        )
                close_and_evict(t, pts)

    removed = _dedup_ldweights(nc, mybir)
    assert removed > 1500, f"ldweights dedup removed only {removed}"
    nc.compile()
    return nc


def _pack_x(x_f32):
    import ml_dtypes
    # x_re[p, T, a, t] = x2[T*128 + t, a*128 + p], per token group
    out = []
    for g in range(T_GRPS):
        x2 = x_f32[g * TOK_SHARD:(g + 1) * TOK_SHARD]
        xr = x2.reshape(T_TILES, 128, K_TILES, 128)      # (T, t, a, p)
        out.append(np.ascontiguousarray(
            xr.transpose(3, 0, 2, 1).astype(ml_dtypes.bfloat16)))
    return out


def kernel(x, weight, A, B, bias):
    global LAST_RESULTS
    import ml_dtypes
    from concourse.bass_utils import run_bass_kernel_spmd

    if "nc" not in _CACHE:
        _CACHE["nc"] = _build_nc()
    nc = _CACHE["nc"]

    weight = np.asarray(weight, dtype=np.float32)
    A = np.asarray(A, dtype=np.float32)
    B = np.asarray(B, dtype=np.float32)
    bias = np.asarray(bias, dtype=np.float32)
    x2 = np.asarray(x, dtype=np.float32).reshape(TOK, IN_F)

    x_parts = _pack_x(x2)
    # u^T = A @ x^T, [16 x TOK] in f32, sliced per token group in bf16
    u_t = A @ x2.T
    u_parts = [np.ascontiguousarray(
        u_t[:, g * TOK_SHARD:(g + 1) * TOK_SHARD].astype(ml_dtypes.bfloat16))
        for g in range(T_GRPS)]

    w_parts, b_parts, bias_parts = [], [], []
    for g in range(O_GRPS):
        sl = slice(g * O_SHARD, (g + 1) * O_SHARD)
        w_s = weight[sl]                                  # (2048, 4096)
        # w_re[p, a, o] = w_s[o, a*128 + p]
        w_parts.append(np.ascontiguousarray(
            w_s.T.reshape(K_TILES, 128, O_SHARD).transpose(1, 0, 2)
            .astype(ml_dtypes.bfloat16)))
        b_parts.append(np.ascontiguousarray(
            B[sl].T.astype(ml_dtypes.bfloat16)))          # (16, 2048)
        bias_parts.append(np.ascontiguousarray(
            np.broadcast_to(bias[sl], (128, O_SHARD))))

    in_maps = []
    for core in range(N_CORES):
        t_grp, o_grp = core // O_GRPS, core % O_GRPS
        in_maps.append({
            "x_re": x_parts[t_grp],
            "w_re": w_parts[o_grp],
            "u_t": u_parts[t_grp],
            "b_t": b_parts[o_grp],
            "bias_b": bias_parts[o_grp],
        })

    res = run_bass_kernel_spmd(nc, in_maps, core_ids=list(range(N_CORES)))
    LAST_RESULTS = res

    y = np.empty((TOK, OUT_F), dtype=np.float32)
    for core in range(N_CORES):
        t_grp, o_grp = core // O_GRPS, core % O_GRPS
        y[t_grp * TOK_SHARD:(t_grp + 1) * TOK_SHARD,
          o_grp * O_SHARD:(o_grp + 1) * O_SHARD] = res.results[core]["y"]
    return y.reshape(B_DIM, S_DIM, OUT_F)
